# revision 28
# baseline (speedup 1.0000x reference)
"""Trainium2 Bass kernel for nn_Attention_1503238553757 (LSA attention).

Reference computation (per batch element):
    qkv = x @ w_qkv; q,k,v heads of dim 64
    dots = (q @ k^T) * scale[h]; diagonal masked to -inf
    attn = softmax(dots); out = attn @ v
    y = concat_heads(out) @ w_out + b_out

Sharding: data-parallel over batch (16 batches -> 2 per core x 8 cores).

Per-core plan (fp16 operands everywhere on the PE; fp32 PSUM):
  - x [1024, 512] loaded token-major fp16, transposed on PE -> xT [512, 1024]
  - qT,kT channel-major via lhsT=w_qkv, rhs=xT    (scoresT needs ch-major)
  - v token-major via lhsT=xT, rhs=w_qkv[:, v]    (attn@V lhsT needs tok-major)
  - scoresT[j, i] = kT_h-slice @ qT_h  (keys on partitions)
  - expT = exp(scale_h * scoresT) via ACT (PSUM->SBUF bf16), diag zeroed via
    affine_select on gpsimd
  - FLASH-STYLE attn@V: all 8 expT j-tiles of a head are kept in SBUF, then
    one 16-matmul burst accumulates [v_h | ones] @ expT -> psO [65, 1024]
    (row 64 = softmax denominators).  Short PSUM residency -> the work pool
    is shared between bursts and projection chunks, so the PE never blocks
    on PSUM held hostage by the slow exp stream.
  - normalize: psO copy -> o_tmp, fast reciprocal of row 64, DRAM-bounce
    partition-broadcast (HWDGE queues), DVE multiply -> osb fp16 (ch-major)
  - y = oT.T @ w_out (+ b_out on gpsimd), token-major, DMA'd out on
    alternating sync/scalar queues

PSUM: psS (scores) 2 x [128,1024] f32 = 4 banks; psW (shared work: qk/v/
transpose/yproj chunks + attn@V bursts) 2 x [128,1024] f32 = 4 banks.

Emission is software-pipelined with a budgeted filler queue: during each
head-pair's scores/exp stream (ACT-bound), the PE executes queued chunks
(other projections, next batch's prep, previous pair's attn@V bursts,
previous batch's output projection).  The last pair of the last batch
streams attn@V per-jt (original style) to minimize the tail.
"""

import os
import sys

for _p in ("/opt/trn_rl_repo", "/root/.axon_site/_ro/trn_rl_repo"):
    if os.path.isdir(_p) and _p not in sys.path:
        sys.path.insert(0, _p)

import numpy as np

import concourse.bass as bass
import concourse.bacc as bacc
import concourse.tile as tile
import concourse.mybir as mybir
from concourse.bass_utils import run_bass_kernel_spmd

# Problem constants (hardcoded per harness contract)
B, N, D = 16, 1024, 512
HEADS, DH = 8, 64
N_CORES = 8
BPC = B // N_CORES  # batches per core = 2

dt = mybir.dt
F32 = dt.float32
BF16 = dt.bfloat16
F16 = dt.float16
ATT_DT = BF16
EXP = mybir.ActivationFunctionType.Exp

NT = N // 128   # token tiles = 8
VW = DH + 1     # per-head v width (v | ones)
KD = D // 128   # d/inner k-tiles = 4


def build_program():
    nc = bacc.Bacc("TRN2", target_bir_lowering=False, debug=False,
                   num_devices=N_CORES)

    x = nc.dram_tensor("x", [BPC, N, D], F32, kind="ExternalInput").ap()
    w_qkv = nc.dram_tensor("w_qkv", [D, 3 * D], F32, kind="ExternalInput").ap()
    w_out = nc.dram_tensor("w_out", [D, D], F32, kind="ExternalInput").ap()
    b_out = nc.dram_tensor("b_out", [D], F32, kind="ExternalInput").ap()
    scale = nc.dram_tensor("scale", [HEADS], F32, kind="ExternalInput").ap()
    y = nc.dram_tensor("y", [BPC, N, D], F32, kind="ExternalOutput").ap()

    ident_dram = nc.inline_tensor(np.eye(128, dtype=np.float16), name="ident")

    import contextlib
    with tile.TileContext(nc) as tc, contextlib.ExitStack() as ctx:
        consts = ctx.enter_context(tc.tile_pool(name="consts", bufs=1))
        p_x = ctx.enter_context(tc.tile_pool(name="p_x", bufs=2))
        p_exp = ctx.enter_context(tc.tile_pool(name="p_exp", bufs=22))
        p_mid = ctx.enter_context(tc.tile_pool(name="p_mid", bufs=3))
        p_qk = ctx.enter_context(tc.tile_pool(name="p_qk", bufs=2))
        p_v = ctx.enter_context(tc.tile_pool(name="p_v", bufs=2))
        p_y = ctx.enter_context(tc.tile_pool(name="p_y", bufs=3))
        p_rb = ctx.enter_context(tc.tile_pool(name="p_rb", bufs=2))
        p_otmp = ctx.enter_context(tc.tile_pool(name="p_otmp", bufs=2))
        p_small = ctx.enter_context(tc.tile_pool(name="p_small", bufs=2))
        psS = ctx.enter_context(tc.tile_pool(name="psS", bufs=2, space="PSUM"))
        psW = ctx.enter_context(tc.tile_pool(name="psW", bufs=2, space="PSUM"))
        p_dram = ctx.enter_context(tc.tile_pool(name="p_dram", bufs=2, space="DRAM"))

        # ---- constants (ident first: transposes only need x + ident) ----
        ident_sb = consts.tile([128, 128], F16)
        nc.sync.dma_start(out=ident_sb, in_=ident_dram.ap())
        wqkv_sb = consts.tile([128, KD, 3 * D], F16)
        wout_sb = consts.tile([128, KD, D], F16)
        bout_bc = consts.tile([128, D], F32)
        scale_sb = consts.tile([128, HEADS], F32)

        wq_src = w_qkv.rearrange("(k p) c -> p k c", p=128)

        def emit_wqkv_ct(ct):
            # one 128-channel column tile of w_qkv (q/k ct 0..7, v 8..11)
            nc.gpsimd.dma_start(
                out=wqkv_sb[:, :, 128 * ct:128 * ct + 128],
                in_=wq_src[:, :, 128 * ct:128 * ct + 128],
            )

        def emit_small_consts():
            nc.gpsimd.dma_start(
                out=scale_sb,
                in_=bass.AP(tensor=scale.tensor, offset=0,
                            ap=[[0, 128], [1, HEADS]]),
            )
            nc.gpsimd.dma_start(
                out=bout_bc,
                in_=bass.AP(tensor=b_out.tensor, offset=0,
                            ap=[[0, 128], [1, D]]),
            )

        def emit_wout_load():
            nc.gpsimd.dma_start(
                out=wout_sb,
                in_=w_out.rearrange("(k p) c -> p k c", p=128),
            )

        # per-batch state kept across the pipelined emission
        xT = [None] * BPC   # each: [tileA, tileB], tile = [128, 2, N] fp16
        qkT = [None] * BPC
        vsb = [None] * BPC
        osb = [None] * BPC
        # expT tiles per (batch, head, jt)
        expt = [[[None] * NT for _ in range(HEADS)] for _ in range(BPC)]

        def xt(b, kt):
            return xT[b][kt // 2][:, kt % 2, :]

        def emit_load_x(b, chunks=2):
            x_sb = p_x.tile([128, NT, D], F16, tag="x", name=f"x_sb{b}")
            src = x[b].rearrange("(r p) d -> p r d", p=128)
            step = NT // chunks
            for c in range(chunks):
                nc.gpsimd.dma_start(out=x_sb[:, c * step:(c + 1) * step, :],
                                    in_=src[:, c * step:(c + 1) * step, :])
            return x_sb

        def emit_transpose_half(b, x_sb, kd, half):
            ps_t = psW.tile([128, 1024], F16, tag="psW",
                            name=f"ps_t_{b}_{kd}_{half}")
            for rr in range(4):
                r = 4 * half + rr
                nc.tensor.transpose(
                    ps_t[:, 128 * rr:128 * rr + 128],
                    x_sb[:, r, 128 * kd:128 * kd + 128],
                    ident_sb,
                )
            nc.vector.tensor_copy(
                xt(b, kd)[:, 512 * half:512 * half + 512], ps_t[:, 0:512]
            )

        def emit_xbar_transposes(b, xscr):
            """x^T via DMA XBAR from a fp16 DRAM scratch copy of x[b]:
            no PE/DVE/PSUM involvement at all."""
            nc.sync.dma_start_transpose(xT[b][0], xscr[:, 0:256])
            nc.scalar.dma_start_transpose(xT[b][1], xscr[:, 256:512])

        def emit_qk_ct(b, ct):
            """One w_qkv column tile (128 channels of q or k) over all tokens:
            8 matmuls -> [128, 1024] PSUM -> one DVE cast to qkT."""
            ps_qk = psW.tile([128, 1024], F32, tag="psW", name=f"ps_qk_{b}_{ct}")
            for nh in range(2):
                for kt in range(KD):
                    nc.tensor.matmul(
                        ps_qk[:, 512 * nh:512 * nh + 512],
                        wqkv_sb[:, kt, 128 * ct:128 * ct + 128],
                        xt(b, kt)[:, 512 * nh:512 * nh + 512],
                        start=(kt == 0), stop=(kt == KD - 1),
                    )
            nc.vector.tensor_copy(qkT[b][:, ct, :], ps_qk)

        def emit_v_r(b, r):
            ps_v = psW.tile([128, 1024], F32, tag="psW", name=f"ps_v_{b}_{r}")
            for kt in range(KD):
                nc.tensor.matmul(
                    ps_v[:, 0:512],
                    xt(b, kt)[:, 128 * r:128 * r + 128],
                    wqkv_sb[:, kt, 2 * D:3 * D],
                    start=(kt == 0), stop=(kt == KD - 1),
                )
            nc.vector.tensor_copy(
                vsb[b][:, r, 0:HEADS * VW].rearrange(
                    "p (h e) -> p h e", h=HEADS)[:, :, 0:DH],
                ps_v[:, 0:512].rearrange("p (h e) -> p h e", h=HEADS),
            )

        def emit_ones(b):
            nc.vector.memset(
                vsb[b][:, :, 0:HEADS * VW].rearrange(
                    "p r (h e) -> p r h e", h=HEADS)[:, :, :, DH:DH + 1],
                1.0,
            )
            nc.vector.memset(vsb[b][:, :, HEADS * VW:], 1.0)

        def emit_scores(b, g, h, jt):
            """scoresT matmuls for (head h, j-tile jt) -> psS tile."""
            q_off = (h % 2) * 64
            ps_s = psS.tile([128, 1024], F32, tag="psS",
                            name=f"ps_s_{b}_{h}_{jt}")
            for ih in range(2):
                nc.tensor.matmul(
                    ps_s[:, 512 * ih:512 * ih + 512],
                    qkT[b][q_off:q_off + 64, 4 + g, 128 * jt:128 * jt + 128],
                    qkT[b][q_off:q_off + 64, g, 512 * ih:512 * ih + 512],
                    start=True, stop=True,
                )
            return ps_s

        def emit_exp(b, h, jt, ps_s):
            expT = p_exp.tile([128, 1024], ATT_DT, tag="exp",
                              name=f"expT_{b}_{h}_{jt}")
            nc.scalar.activation(expT, ps_s, EXP, scale=scale_sb[:, h:h + 1])
            nc.gpsimd.affine_select(
                out=expT[:, 128 * jt:128 * jt + 128],
                in_=expT[:, 128 * jt:128 * jt + 128],
                compare_op=mybir.AluOpType.not_equal,
                fill=0.0, base=0, channel_multiplier=1,
                pattern=[[-1, 128]],
            )
            expt[b][h][jt] = expT

        def emit_normalize(b, h, src, pe_norm=False):
            """src = [65, 1024] (unnormalized o^T | sums row) in SBUF or PSUM.
            Produces osb[b] rows for head h.  pe_norm: partition-broadcast the
            reciprocal with a K=1 fp32r matmul into freed scores PSUM instead
            of the 2-hop DRAM bounce (lower latency; used for the tail)."""
            g, q_off = h // 2, (h % 2) * 64
            sums = p_small.tile([1, N], F32, tag="sums", name=f"sums_{b}_{h}")
            nc.vector.tensor_copy(sums, src[DH:DH + 1, :])
            recip = p_small.tile([1, N], F32, tag="recip", name=f"recip_{b}_{h}")
            nc.vector.reciprocal_approx_fast(recip, sums)
            if pe_norm:
                recip16 = p_small.tile([1, N], F16, tag="recip16",
                                       name=f"recip16_{b}_{h}")
                nc.vector.tensor_copy(recip16, recip)
                rbps = psS.tile([64, N], F32, tag="psS", name=f"rbps_{b}_{h}")
                for ih in range(2):
                    nc.tensor.matmul(
                        rbps[:, 512 * ih:512 * ih + 512],
                        ones_row,
                        recip16[:, 512 * ih:512 * ih + 512],
                        start=True, stop=True,
                    )
                rb = rbps
            else:
                scr = p_dram.tile([1, N], F32, tag="scr", name=f"scr_{b}_{h}")
                nc.sync.dma_start(out=scr, in_=recip)
                rb = p_rb.tile([64, N], F32, tag="rb", name=f"rb_{b}_{h}")
                nc.gpsimd.dma_start(
                    out=rb,
                    in_=bass.AP(tensor=scr.tensor, offset=scr.offset,
                                ap=[[0, 64], [1, N]]),
                )
            nc.vector.tensor_mul(
                osb[b][q_off:q_off + 64, g, :], src[0:DH, :], rb
            )

        def emit_burst(b, h, pe_norm=False):
            """Flash-style attn@V for head h: 16 back-to-back matmuls over all
            retained expT tiles, then normalize via an o_tmp bounce."""
            ps_o = psW.tile([DH + 1, 1024], F32, tag="psW",
                            name=f"ps_o_{b}_{h}")
            for jt in range(NT):
                eT = expt[b][h][jt]
                for ih in range(2):
                    nc.tensor.matmul(
                        ps_o[:, 512 * ih:512 * ih + 512],
                        vsb[b][:, jt, VW * h:VW * h + DH + 1],
                        eT[:, 512 * ih:512 * ih + 512],
                        start=(jt == 0), stop=(jt == NT - 1),
                    )
            o_tmp = p_otmp.tile([DH + 1, N], F32, tag="otmp",
                                name=f"o_tmp_{b}_{h}")
            nc.vector.tensor_copy(o_tmp, ps_o)
            emit_normalize(b, h, o_tmp, pe_norm=pe_norm)

        ydma = [0]

        def emit_yproj_r(b, r):
            ps_y = psW.tile([128, 1024], F32, tag="psW", name=f"ps_y_{b}_{r}")
            for kt in range(KD):
                nc.tensor.matmul(
                    ps_y[:, 0:512],
                    osb[b][:, kt, 128 * r:128 * r + 128],
                    wout_sb[:, kt, :],
                    start=(kt == 0), stop=(kt == KD - 1),
                )
            y_sb = p_y.tile([128, D], F32, tag="y")
            nc.vector.tensor_add(y_sb, ps_y[:, 0:512], bout_bc)
            eng = nc.sync if ydma[0] % 2 == 0 else nc.scalar
            ydma[0] += 1
            eng.dma_start(out=y[b, 128 * r:128 * r + 128, :], in_=y_sb)

        # ================= budgeted filler queue =================
        # Each entry: (cost_us, closure, label).  Popped between scores/exp
        # chunks at the PE's spare-capacity rate; force() drains through a
        # label when later emissions depend on it (emission order IS
        # dependency order for the in-order engines).
        import functools
        F = functools.partial
        fillers = []
        budget = [0.0]

        def q(cost, fn, label=None):
            fillers.append((cost, fn, label))

        def pop_fillers(us):
            budget[0] += us
            while fillers and budget[0] >= fillers[0][0]:
                cost, fn, _ = fillers.pop(0)
                budget[0] -= cost
                fn()

        def force(label):
            while fillers:
                cost, fn, lab = fillers.pop(0)
                fn()
                if lab == label:
                    return

        def drain_fillers():
            while fillers:
                fillers.pop(0)[1]()
            budget[0] = 0.0

        def emit_pair(b, g, streaming=False):
            """Scores+exp stream for pair g of batch b, with fillers popped
            at a rate matched to the PE's spare capacity under the
            ACT-bound exp stream.  streaming=True: accumulate attn@V
            per-jt (tail-latency mode for the final pair)."""
            heads = (2 * g, 2 * g + 1)
            ps_os = {}
            if streaming:
                for h in heads:
                    ps_os[h] = psW.tile([DH + 1, N], F32, tag="psW",
                                        name=f"ps_os_{b}_{h}")
            for jt in range(NT):
                # both heads' score matmuls adjacent: h0 occupies PE rows
                # 0-63, h1 rows 64-127 (disjoint tiles)
                ps_list = [emit_scores(b, g, h, jt) for h in heads]
                for h, ps_s in zip(heads, ps_list):
                    emit_exp(b, h, jt, ps_s)
                    if streaming:
                        eT = expt[b][h][jt]
                        for ih in range(2):
                            nc.tensor.matmul(
                                ps_os[h][:, 512 * ih:512 * ih + 512],
                                vsb[b][:, jt, VW * h:VW * h + DH + 1],
                                eT[:, 512 * ih:512 * ih + 512],
                                start=(jt == 0), stop=(jt == NT - 1),
                            )
                    else:
                        pop_fillers(0.85)  # PE spare per exp (2.4us ACT-bound
                        #                    jt minus scores+overhead)
            if streaming:
                for h in heads:
                    o_tmp = p_otmp.tile([DH + 1, N], F32, tag="otmp",
                                        name=f"o_tmp_s_{b}_{h}")
                    nc.vector.tensor_copy(o_tmp, ps_os[h])
                    emit_normalize(b, h, o_tmp, pe_norm=True)

        # ================= pipelined emission =================
        # Prologue: minimal path to the first scores matmul.  gpsimd DMA
        # issue order: x0 chunks, small consts, then w_qkv column tiles in
        # the order projections consume them (the full-w load was serializing
        # the first qk projection ~20us behind x).
        x0 = emit_load_x(0, chunks=4)
        emit_small_consts()
        emit_wqkv_ct(0)
        emit_wqkv_ct(4)
        xT[0] = [p_mid.tile([128, 2, N], F16, tag="xt", bufs=4, name="xT0a"),
                 p_mid.tile([128, 2, N], F16, tag="xt", bufs=4, name="xT0b")]
        qkT[0] = p_qk.tile([128, 8, N], F16, tag="qk", name="qkT0")
        vsb[0] = p_v.tile([128, NT, HEADS * VW + 64], ATT_DT, tag="v", name="v0")
        ones_row = consts.tile([1, 64], F16)
        nc.vector.memset(ones_row, 1.0)
        for half in range(2):
            for kd in range(KD):
                emit_transpose_half(0, x0, kd, half)
        emit_qk_ct(0, 0)       # q heads 0,1
        emit_qk_ct(0, 4)       # k heads 0,1
        emit_ones(0)

        # C(0,0) fillers: v tiles (needed by bursts in C(0,1)), pair-1 qk,
        # then the rest of B(0) and the start of B(1).
        def start_b1():
            # batch 1 x^T comes straight off the DMA XBAR: f32->f16 cast to a
            # DRAM scratch, then two transposing reads; zero PE/DVE work.
            xscr = p_dram.tile([N, D], F16, tag="xscr", name="xscr1")
            nc.gpsimd.dma_start(out=xscr, in_=x[1])
            xT[1] = [p_mid.tile([128, 2, N], F16, tag="xt", bufs=4, name="xT1a"),
                     p_mid.tile([128, 2, N], F16, tag="xt", bufs=4, name="xT1b")]
            qkT[1] = p_qk.tile([128, 8, N], F16, tag="qk", name="qkT1")
            vsb[1] = p_v.tile([128, NT, HEADS * VW + 64], ATT_DT, tag="v",
                              name="v1")
            emit_xbar_transposes(1, xscr)

        osb[0] = p_mid.tile([128, KD, N], F16, tag="mid", bufs=2, name="o0")

        # C(0,0) fillers: v weights + v tiles (bursts in C(0,1) need them
        # all), pair-1 qk.
        for ct in (8, 9, 10, 11):
            q(0.0, F(emit_wqkv_ct, ct))
        q(0.0, F(emit_wqkv_ct, 1))
        q(0.0, F(emit_wqkv_ct, 5))
        for r in range(NT):
            q(1.7, F(emit_v_r, 0, r), "v0" if r == NT - 1 else None)
        q(3.4, F(emit_qk_ct, 0, 1))
        q(3.4, F(emit_qk_ct, 0, 5), "qk01")
        emit_pair(0, 0)

        # C(0,1): bursts for pair 0 first (free p_exp slots), then B(0)
        # leftovers and the start of B(1).
        force("v0")        # bursts read all of vsb[0]
        force("qk01")      # pair(0,1) scores need ct1/ct5
        fillers.insert(0, (3.4, F(emit_burst, 0, 0), None))
        fillers.insert(1, (3.4, F(emit_burst, 0, 1), None))
        q(0.0, F(emit_wqkv_ct, 2))
        q(0.0, F(emit_wqkv_ct, 6))
        q(3.4, F(emit_qk_ct, 0, 2))
        q(3.4, F(emit_qk_ct, 0, 6), "qk02")
        q(0.2, start_b1)
        q(0.0, emit_wout_load)
        emit_pair(0, 1)

        force("qk02")
        fillers.insert(0, (3.4, F(emit_burst, 0, 2), None))
        fillers.insert(1, (3.4, F(emit_burst, 0, 3), None))
        q(0.0, F(emit_wqkv_ct, 3))
        q(0.0, F(emit_wqkv_ct, 7))
        q(3.4, F(emit_qk_ct, 0, 3))
        q(3.4, F(emit_qk_ct, 0, 7), "qk03")
        q(0.3, F(emit_ones, 1))
        emit_pair(0, 2)

        force("qk03")
        fillers.insert(0, (3.4, F(emit_burst, 0, 4), None))
        fillers.insert(1, (3.4, F(emit_burst, 0, 5), None))
        q(3.4, F(emit_qk_ct, 1, 0))
        q(3.4, F(emit_qk_ct, 1, 4), "qk10")
        emit_pair(0, 3)

        force("qk10")      # pair(1,0) scores need b1 ct0/ct4
        fillers.insert(0, (3.4, F(emit_burst, 0, 6), None))
        fillers.insert(1, (3.4, F(emit_burst, 0, 7), None))
        for r in range(NT):
            q(1.7, F(emit_v_r, 1, r), "v1" if r == NT - 1 else None)
        q(3.4, F(emit_qk_ct, 1, 1))
        q(3.4, F(emit_qk_ct, 1, 5), "qk11")
        emit_pair(1, 0)

        osb[1] = p_mid.tile([128, KD, N], F16, tag="mid", bufs=2, name="o1")

        force("v1")
        force("qk11")
        fillers.insert(0, (3.4, F(emit_burst, 1, 0), None))
        fillers.insert(1, (3.4, F(emit_burst, 1, 1), None))
        q(3.4, F(emit_qk_ct, 1, 2))
        q(3.4, F(emit_qk_ct, 1, 6), "qk12")
        q(1.9, F(emit_yproj_r, 0, 0))
        q(1.9, F(emit_yproj_r, 0, 1))
        emit_pair(1, 1)

        force("qk12")
        fillers.insert(0, (3.4, F(emit_burst, 1, 2), None))
        fillers.insert(1, (3.4, F(emit_burst, 1, 3), None))
        q(3.4, F(emit_qk_ct, 1, 3))
        q(3.4, F(emit_qk_ct, 1, 7), "qk13")
        for r in range(2, 5):
            q(1.9, F(emit_yproj_r, 0, r))
        emit_pair(1, 2)

        force("qk13")
        emit_burst(1, 4)
        emit_burst(1, 5)
        for r in range(5, 8):
            q(1.9, F(emit_yproj_r, 0, r))
        drain_fillers()
        # Last pair streams attn@V inline and normalizes via the PE
        # broadcast: after the final exp only normalize + D(1) remain.
        emit_pair(1, 3, streaming=True)

        for r in range(NT):
            emit_yproj_r(1, r)

    nc.compile()
    return nc


_NC = None


def _get_program():
    global _NC
    if _NC is None:
        _NC = build_program()
    return _NC


def make_in_maps(x, w_qkv, w_out, b_out, scale):
    x = np.ascontiguousarray(np.asarray(x, dtype=np.float32))
    w_qkv = np.ascontiguousarray(np.asarray(w_qkv, dtype=np.float32))
    w_out = np.ascontiguousarray(np.asarray(w_out, dtype=np.float32))
    b_out = np.ascontiguousarray(np.asarray(b_out, dtype=np.float32))
    scale = np.ascontiguousarray(np.asarray(scale, dtype=np.float32))
    return [
        {
            "x": x[c * BPC:(c + 1) * BPC],
            "w_qkv": w_qkv,
            "w_out": w_out,
            "b_out": b_out,
            "scale": scale,
        }
        for c in range(N_CORES)
    ]


def kernel(x, w_qkv, w_out, b_out, scale):
    nc = _get_program()
    in_maps = make_in_maps(x, w_qkv, w_out, b_out, scale)
    res = run_bass_kernel_spmd(nc, in_maps, core_ids=list(range(N_CORES)))
    return np.concatenate([res.results[c]["y"] for c in range(N_CORES)], axis=0)


if __name__ == "__main__":
    rng = np.random.default_rng(0)
    inputs = {
        "x": rng.standard_normal((B, N, D), dtype=np.float32),
        "w_qkv": rng.standard_normal((D, 3 * D), dtype=np.float32) * 0.03,
        "w_out": rng.standard_normal((D, D), dtype=np.float32) * 0.04,
        "b_out": np.zeros(D, dtype=np.float32),
        "scale": np.full(HEADS, DH ** -0.5, dtype=np.float32),
    }
    out = kernel(**inputs)
    print("kernel output", out.shape, out.dtype)


# revision 32
# speedup vs baseline: 1.0329x; 1.0329x over previous
"""Trainium2 Bass kernel for nn_Attention_1503238553757 (LSA attention).

Reference computation (per batch element):
    qkv = x @ w_qkv; q,k,v heads of dim 64
    dots = (q @ k^T) * scale[h]; diagonal masked to -inf
    attn = softmax(dots); out = attn @ v
    y = concat_heads(out) @ w_out + b_out

Sharding: data-parallel over batch (16 batches -> 2 per core x 8 cores).

Per-core plan (fp16 operands everywhere on the PE; fp32 PSUM):
  - x [1024, 512] loaded token-major fp16, transposed on PE -> xT [512, 1024]
  - qT,kT channel-major via lhsT=w_qkv, rhs=xT    (scoresT needs ch-major)
  - v token-major via lhsT=xT, rhs=w_qkv[:, v]    (attn@V lhsT needs tok-major)
  - scoresT[j, i] = kT_h-slice @ qT_h  (keys on partitions)
  - expT = exp(scale_h * scoresT) via ACT (PSUM->SBUF bf16), diag zeroed via
    affine_select on gpsimd
  - FLASH-STYLE attn@V: all 8 expT j-tiles of a head are kept in SBUF, then
    one 16-matmul burst accumulates [v_h | ones] @ expT -> psO [65, 1024]
    (row 64 = softmax denominators).  Short PSUM residency -> the work pool
    is shared between bursts and projection chunks, so the PE never blocks
    on PSUM held hostage by the slow exp stream.
  - normalize: psO copy -> o_tmp, fast reciprocal of row 64, DRAM-bounce
    partition-broadcast (HWDGE queues), DVE multiply -> osb fp16 (ch-major)
  - y = oT.T @ w_out (+ b_out on gpsimd), token-major, DMA'd out on
    alternating sync/scalar queues

PSUM: psS (scores) 2 x [128,1024] f32 = 4 banks; psW (shared work: qk/v/
transpose/yproj chunks + attn@V bursts) 2 x [128,1024] f32 = 4 banks.

Emission is software-pipelined with a budgeted filler queue: during each
head-pair's scores/exp stream (ACT-bound), the PE executes queued chunks
(other projections, next batch's prep, previous pair's attn@V bursts,
previous batch's output projection).  The last pair of the last batch
streams attn@V per-jt (original style) to minimize the tail.
"""

import os
import sys

for _p in ("/opt/trn_rl_repo", "/root/.axon_site/_ro/trn_rl_repo"):
    if os.path.isdir(_p) and _p not in sys.path:
        sys.path.insert(0, _p)

import numpy as np

import concourse.bass as bass
import concourse.bacc as bacc
import concourse.tile as tile
import concourse.mybir as mybir
from concourse.bass_utils import run_bass_kernel_spmd

# Problem constants (hardcoded per harness contract)
B, N, D = 16, 1024, 512
HEADS, DH = 8, 64
N_CORES = 8
BPC = B // N_CORES  # batches per core = 2

dt = mybir.dt
F32 = dt.float32
BF16 = dt.bfloat16
F16 = dt.float16
ATT_DT = BF16
EXP = mybir.ActivationFunctionType.Exp

NT = N // 128   # token tiles = 8
VW = DH + 1     # per-head v width (v | ones)
KD = D // 128   # d/inner k-tiles = 4


def build_program():
    nc = bacc.Bacc("TRN2", target_bir_lowering=False, debug=False,
                   num_devices=N_CORES)

    x = nc.dram_tensor("x", [BPC, N, D], F32, kind="ExternalInput").ap()
    w_qkv = nc.dram_tensor("w_qkv", [D, 3 * D], F32, kind="ExternalInput").ap()
    w_out = nc.dram_tensor("w_out", [D, D], F32, kind="ExternalInput").ap()
    b_out = nc.dram_tensor("b_out", [D], F32, kind="ExternalInput").ap()
    scale = nc.dram_tensor("scale", [HEADS], F32, kind="ExternalInput").ap()
    y = nc.dram_tensor("y", [BPC, N, D], F32, kind="ExternalOutput").ap()

    ident_dram = nc.inline_tensor(np.eye(128, dtype=np.float16), name="ident")

    import contextlib
    with tile.TileContext(nc) as tc, contextlib.ExitStack() as ctx:
        consts = ctx.enter_context(tc.tile_pool(name="consts", bufs=1))
        p_x = ctx.enter_context(tc.tile_pool(name="p_x", bufs=2))
        p_exp = ctx.enter_context(tc.tile_pool(name="p_exp", bufs=22))
        p_mid = ctx.enter_context(tc.tile_pool(name="p_mid", bufs=3))
        p_qk = ctx.enter_context(tc.tile_pool(name="p_qk", bufs=2))
        p_v = ctx.enter_context(tc.tile_pool(name="p_v", bufs=2))
        p_y = ctx.enter_context(tc.tile_pool(name="p_y", bufs=3))
        p_rb = ctx.enter_context(tc.tile_pool(name="p_rb", bufs=2))
        p_otmp = ctx.enter_context(tc.tile_pool(name="p_otmp", bufs=2))
        p_small = ctx.enter_context(tc.tile_pool(name="p_small", bufs=2))
        psS = ctx.enter_context(tc.tile_pool(name="psS", bufs=2, space="PSUM"))
        psW = ctx.enter_context(tc.tile_pool(name="psW", bufs=2, space="PSUM"))
        p_dram = ctx.enter_context(tc.tile_pool(name="p_dram", bufs=2, space="DRAM"))

        # ---- constants (ident first: transposes only need x + ident) ----
        ident_sb = consts.tile([128, 128], F16)
        nc.sync.dma_start(out=ident_sb, in_=ident_dram.ap())
        wqkv_sb = consts.tile([128, KD, 3 * D], F16)
        wout_sb = consts.tile([128, KD, D], F16)
        bout_bc = consts.tile([128, D], F32)
        scale_sb = consts.tile([128, HEADS], F32)

        wq_src = w_qkv.rearrange("(k p) c -> p k c", p=128)

        def emit_wqkv_ct(ct):
            # one 128-channel column tile of w_qkv (q/k ct 0..7, v 8..11)
            nc.gpsimd.dma_start(
                out=wqkv_sb[:, :, 128 * ct:128 * ct + 128],
                in_=wq_src[:, :, 128 * ct:128 * ct + 128],
            )

        def emit_small_consts():
            nc.gpsimd.dma_start(
                out=scale_sb,
                in_=bass.AP(tensor=scale.tensor, offset=0,
                            ap=[[0, 128], [1, HEADS]]),
            )
            nc.gpsimd.dma_start(
                out=bout_bc,
                in_=bass.AP(tensor=b_out.tensor, offset=0,
                            ap=[[0, 128], [1, D]]),
            )

        def emit_wout_load():
            nc.gpsimd.dma_start(
                out=wout_sb,
                in_=w_out.rearrange("(k p) c -> p k c", p=128),
            )

        # per-batch state kept across the pipelined emission
        xT = [None] * BPC   # each: [tileA, tileB], tile = [128, 2, N] fp16
        qkT = [None] * BPC
        vsb = [None] * BPC
        osb = [None] * BPC
        # expT tiles per (batch, head, jt)
        expt = [[[None] * NT for _ in range(HEADS)] for _ in range(BPC)]

        def xt(b, kt):
            return xT[b][kt // 2][:, kt % 2, :]

        def emit_load_x(b, chunks=2):
            x_sb = p_x.tile([128, NT, D], F16, tag="x", name=f"x_sb{b}")
            src = x[b].rearrange("(r p) d -> p r d", p=128)
            step = NT // chunks
            for c in range(chunks):
                nc.gpsimd.dma_start(out=x_sb[:, c * step:(c + 1) * step, :],
                                    in_=src[:, c * step:(c + 1) * step, :])
            return x_sb

        def emit_transpose_half(b, x_sb, kd, half):
            ps_t = psW.tile([128, 1024], F16, tag="psW",
                            name=f"ps_t_{b}_{kd}_{half}")
            for rr in range(4):
                r = 4 * half + rr
                nc.tensor.transpose(
                    ps_t[:, 128 * rr:128 * rr + 128],
                    x_sb[:, r, 128 * kd:128 * kd + 128],
                    ident_sb,
                )
            nc.vector.tensor_copy(
                xt(b, kd)[:, 512 * half:512 * half + 512], ps_t[:, 0:512]
            )

        def emit_xbar_transposes(b, xscr):
            """x^T via DMA XBAR from a fp16 DRAM scratch copy of x[b]:
            no PE/DVE/PSUM involvement at all."""
            nc.sync.dma_start_transpose(xT[b][0], xscr[:, 0:256])
            nc.scalar.dma_start_transpose(xT[b][1], xscr[:, 256:512])

        def emit_qk_ct(b, ct):
            """One w_qkv column tile (128 channels of q or k) over all tokens:
            8 matmuls -> [128, 1024] PSUM -> one DVE cast to qkT."""
            ps_qk = psW.tile([128, 1024], F32, tag="psW", name=f"ps_qk_{b}_{ct}")
            for nh in range(2):
                for kt in range(KD):
                    nc.tensor.matmul(
                        ps_qk[:, 512 * nh:512 * nh + 512],
                        wqkv_sb[:, kt, 128 * ct:128 * ct + 128],
                        xt(b, kt)[:, 512 * nh:512 * nh + 512],
                        start=(kt == 0), stop=(kt == KD - 1),
                    )
            nc.vector.tensor_copy(qkT[b][:, ct, :], ps_qk)

        def emit_v_r(b, r):
            ps_v = psW.tile([128, 1024], F32, tag="psW", name=f"ps_v_{b}_{r}")
            for kt in range(KD):
                nc.tensor.matmul(
                    ps_v[:, 0:512],
                    xt(b, kt)[:, 128 * r:128 * r + 128],
                    wqkv_sb[:, kt, 2 * D:3 * D],
                    start=(kt == 0), stop=(kt == KD - 1),
                )
            nc.vector.tensor_copy(
                vsb[b][:, r, 0:HEADS * VW].rearrange(
                    "p (h e) -> p h e", h=HEADS)[:, :, 0:DH],
                ps_v[:, 0:512].rearrange("p (h e) -> p h e", h=HEADS),
            )

        def emit_ones(b):
            nc.vector.memset(
                vsb[b][:, :, 0:HEADS * VW].rearrange(
                    "p r (h e) -> p r h e", h=HEADS)[:, :, :, DH:DH + 1],
                1.0,
            )
            nc.vector.memset(vsb[b][:, :, HEADS * VW:], 1.0)

        def emit_scores(b, g, h, jt):
            """scoresT matmuls for (head h, j-tile jt) -> psS tile."""
            q_off = (h % 2) * 64
            ps_s = psS.tile([128, 1024], F32, tag="psS",
                            name=f"ps_s_{b}_{h}_{jt}")
            for ih in range(2):
                nc.tensor.matmul(
                    ps_s[:, 512 * ih:512 * ih + 512],
                    qkT[b][q_off:q_off + 64, 4 + g, 128 * jt:128 * jt + 128],
                    qkT[b][q_off:q_off + 64, g, 512 * ih:512 * ih + 512],
                    start=True, stop=True,
                )
            return ps_s

        def emit_exp(b, h, jt, ps_s):
            expT = p_exp.tile([128, 1024], ATT_DT, tag="exp",
                              name=f"expT_{b}_{h}_{jt}")
            nc.scalar.activation(expT, ps_s, EXP, scale=scale_sb[:, h:h + 1])
            nc.gpsimd.affine_select(
                out=expT[:, 128 * jt:128 * jt + 128],
                in_=expT[:, 128 * jt:128 * jt + 128],
                compare_op=mybir.AluOpType.not_equal,
                fill=0.0, base=0, channel_multiplier=1,
                pattern=[[-1, 128]],
            )
            expt[b][h][jt] = expT

        def emit_normalize(b, h, src, pe_norm=False):
            """src = [65, 1024] (unnormalized o^T | sums row) in SBUF or PSUM.
            Produces osb[b] rows for head h.  pe_norm: partition-broadcast the
            reciprocal with a K=1 fp32r matmul into freed scores PSUM instead
            of the 2-hop DRAM bounce (lower latency; used for the tail)."""
            g, q_off = h // 2, (h % 2) * 64
            sums = p_small.tile([1, N], F32, tag="sums", name=f"sums_{b}_{h}")
            nc.vector.tensor_copy(sums, src[DH:DH + 1, :])
            recip = p_small.tile([1, N], F32, tag="recip", name=f"recip_{b}_{h}")
            nc.vector.reciprocal_approx_fast(recip, sums)
            if pe_norm:
                recip16 = p_small.tile([1, N], F16, tag="recip16",
                                       name=f"recip16_{b}_{h}")
                nc.vector.tensor_copy(recip16, recip)
                rbps = psS.tile([64, N], F32, tag="psS", name=f"rbps_{b}_{h}")
                for ih in range(2):
                    nc.tensor.matmul(
                        rbps[:, 512 * ih:512 * ih + 512],
                        ones_row,
                        recip16[:, 512 * ih:512 * ih + 512],
                        start=True, stop=True,
                    )
                rb = rbps
            else:
                scr = p_dram.tile([1, N], F32, tag="scr", name=f"scr_{b}_{h}")
                nc.sync.dma_start(out=scr, in_=recip)
                rb = p_rb.tile([64, N], F32, tag="rb", name=f"rb_{b}_{h}")
                nc.gpsimd.dma_start(
                    out=rb,
                    in_=bass.AP(tensor=scr.tensor, offset=scr.offset,
                                ap=[[0, 64], [1, N]]),
                )
            nc.vector.tensor_mul(
                osb[b][q_off:q_off + 64, g, :], src[0:DH, :], rb
            )

        def emit_burst(b, h, pe_norm=False):
            """Flash-style attn@V for head h: 16 back-to-back matmuls over all
            retained expT tiles, then normalize via an o_tmp bounce."""
            ps_o = psW.tile([DH + 1, 1024], F32, tag="psW",
                            name=f"ps_o_{b}_{h}")
            for jt in range(NT):
                eT = expt[b][h][jt]
                for ih in range(2):
                    nc.tensor.matmul(
                        ps_o[:, 512 * ih:512 * ih + 512],
                        vsb[b][:, jt, VW * h:VW * h + DH + 1],
                        eT[:, 512 * ih:512 * ih + 512],
                        start=(jt == 0), stop=(jt == NT - 1),
                    )
            o_tmp = p_otmp.tile([DH + 1, N], F32, tag="otmp",
                                name=f"o_tmp_{b}_{h}")
            nc.vector.tensor_copy(o_tmp, ps_o)
            emit_normalize(b, h, o_tmp, pe_norm=pe_norm)

        ydma = [0]

        def emit_yproj_r(b, r):
            ps_y = psW.tile([128, 1024], F32, tag="psW", name=f"ps_y_{b}_{r}")
            for kt in range(KD):
                nc.tensor.matmul(
                    ps_y[:, 0:512],
                    osb[b][:, kt, 128 * r:128 * r + 128],
                    wout_sb[:, kt, :],
                    start=(kt == 0), stop=(kt == KD - 1),
                )
            y_sb = p_y.tile([128, D], F32, tag="y")
            nc.vector.tensor_add(y_sb, ps_y[:, 0:512], bout_bc)
            eng = nc.sync if ydma[0] % 2 == 0 else nc.scalar
            ydma[0] += 1
            eng.dma_start(out=y[b, 128 * r:128 * r + 128, :], in_=y_sb)

        # ================= budgeted filler queue =================
        # Each entry: (cost_us, closure, label).  Popped between scores/exp
        # chunks at the PE's spare-capacity rate; force() drains through a
        # label when later emissions depend on it (emission order IS
        # dependency order for the in-order engines).
        import functools
        F = functools.partial
        fillers = []
        budget = [0.0]

        def q(cost, fn, label=None):
            fillers.append((cost, fn, label))

        def pop_fillers(us):
            budget[0] += us
            while fillers and budget[0] >= fillers[0][0]:
                cost, fn, _ = fillers.pop(0)
                budget[0] -= cost
                fn()

        def force(label):
            while fillers:
                cost, fn, lab = fillers.pop(0)
                fn()
                if lab == label:
                    return

        def drain_fillers():
            while fillers:
                fillers.pop(0)[1]()
            budget[0] = 0.0

        def emit_pair(b, g, streaming=False):
            """Scores+exp stream for pair g of batch b, with fillers popped
            at a rate matched to the PE's spare capacity under the
            ACT-bound exp stream.  streaming=True: accumulate attn@V
            per-jt (tail-latency mode for the final pair)."""
            heads = (2 * g, 2 * g + 1)
            ps_os = {}
            if streaming:
                for h in heads:
                    ps_os[h] = psW.tile([DH + 1, N], F32, tag="psW",
                                        name=f"ps_os_{b}_{h}")
            for jt in range(NT):
                for h in heads:
                    ps_s = emit_scores(b, g, h, jt)
                    emit_exp(b, h, jt, ps_s)
                    if streaming:
                        eT = expt[b][h][jt]
                        for ih in range(2):
                            nc.tensor.matmul(
                                ps_os[h][:, 512 * ih:512 * ih + 512],
                                vsb[b][:, jt, VW * h:VW * h + DH + 1],
                                eT[:, 512 * ih:512 * ih + 512],
                                start=(jt == 0), stop=(jt == NT - 1),
                            )
                    else:
                        pop_fillers(0.85)  # PE spare per exp (2.4us ACT-bound
                        #                    jt minus scores+overhead)
            if streaming:
                for h in heads:
                    o_tmp = p_otmp.tile([DH + 1, N], F32, tag="otmp",
                                        name=f"o_tmp_s_{b}_{h}")
                    nc.vector.tensor_copy(o_tmp, ps_os[h])
                    emit_normalize(b, h, o_tmp, pe_norm=True)

        # ================= pipelined emission =================
        # Prologue: minimal path to the first scores matmul.  gpsimd DMA
        # issue order: x0 chunks, small consts, then w_qkv column tiles in
        # the order projections consume them (the full-w load was serializing
        # the first qk projection ~20us behind x).
        x0 = emit_load_x(0, chunks=4)
        emit_small_consts()
        emit_wqkv_ct(0)
        emit_wqkv_ct(4)
        xT[0] = [p_mid.tile([128, 2, N], F16, tag="xt", bufs=4, name="xT0a"),
                 p_mid.tile([128, 2, N], F16, tag="xt", bufs=4, name="xT0b")]
        qkT[0] = p_qk.tile([128, 8, N], F16, tag="qk", name="qkT0")
        vsb[0] = p_v.tile([128, NT, HEADS * VW + 64], ATT_DT, tag="v", name="v0")
        ones_row = consts.tile([1, 64], F16)
        nc.vector.memset(ones_row, 1.0)
        for half in range(2):
            for kd in range(KD):
                emit_transpose_half(0, x0, kd, half)
        emit_qk_ct(0, 0)       # q heads 0,1
        emit_qk_ct(0, 4)       # k heads 0,1
        emit_ones(0)

        # C(0,0) fillers: v tiles (needed by bursts in C(0,1)), pair-1 qk,
        # then the rest of B(0) and the start of B(1).
        def start_b1():
            # batch 1 x^T comes straight off the DMA XBAR: f32->f16 cast to a
            # DRAM scratch, then two transposing reads; zero PE/DVE work.
            xscr = p_dram.tile([N, D], F16, tag="xscr", name="xscr1")
            nc.gpsimd.dma_start(out=xscr, in_=x[1])
            xT[1] = [p_mid.tile([128, 2, N], F16, tag="xt", bufs=4, name="xT1a"),
                     p_mid.tile([128, 2, N], F16, tag="xt", bufs=4, name="xT1b")]
            qkT[1] = p_qk.tile([128, 8, N], F16, tag="qk", name="qkT1")
            vsb[1] = p_v.tile([128, NT, HEADS * VW + 64], ATT_DT, tag="v",
                              name="v1")
            emit_xbar_transposes(1, xscr)

        osb[0] = p_mid.tile([128, KD, N], F16, tag="mid", bufs=2, name="o0")

        # C(0,0) fillers: v weights + v tiles (bursts in C(0,1) need them
        # all), pair-1 qk.
        for ct in (8, 9, 10, 11):
            q(0.0, F(emit_wqkv_ct, ct))
        q(0.0, F(emit_wqkv_ct, 1))
        q(0.0, F(emit_wqkv_ct, 5))
        for r in range(NT):
            q(1.7, F(emit_v_r, 0, r), "v0" if r == NT - 1 else None)
        q(3.4, F(emit_qk_ct, 0, 1))
        q(3.4, F(emit_qk_ct, 0, 5), "qk01")
        emit_pair(0, 0)

        # C(0,1): bursts for pair 0 first (free p_exp slots), then B(0)
        # leftovers and the start of B(1).
        force("v0")        # bursts read all of vsb[0]
        force("qk01")      # pair(0,1) scores need ct1/ct5
        fillers.insert(0, (3.4, F(emit_burst, 0, 0), None))
        fillers.insert(1, (3.4, F(emit_burst, 0, 1), None))
        q(0.0, F(emit_wqkv_ct, 2))
        q(0.0, F(emit_wqkv_ct, 6))
        q(3.4, F(emit_qk_ct, 0, 2))
        q(3.4, F(emit_qk_ct, 0, 6), "qk02")
        q(0.2, start_b1)
        q(0.0, emit_wout_load)
        emit_pair(0, 1)

        force("qk02")
        fillers.insert(0, (3.4, F(emit_burst, 0, 2), None))
        fillers.insert(1, (3.4, F(emit_burst, 0, 3), None))
        q(0.0, F(emit_wqkv_ct, 3))
        q(0.0, F(emit_wqkv_ct, 7))
        q(3.4, F(emit_qk_ct, 0, 3))
        q(3.4, F(emit_qk_ct, 0, 7), "qk03")
        q(0.3, F(emit_ones, 1))
        emit_pair(0, 2)

        force("qk03")
        fillers.insert(0, (3.4, F(emit_burst, 0, 4), None))
        fillers.insert(1, (3.4, F(emit_burst, 0, 5), None))
        q(3.4, F(emit_qk_ct, 1, 0))
        q(3.4, F(emit_qk_ct, 1, 4), "qk10")
        for r in range(4):
            q(1.7, F(emit_v_r, 1, r))
        emit_pair(0, 3)

        force("qk10")      # pair(1,0) scores need b1 ct0/ct4
        fillers.insert(0, (3.4, F(emit_burst, 0, 6), None))
        fillers.insert(1, (3.4, F(emit_burst, 0, 7), None))
        for r in range(4, NT):
            q(1.7, F(emit_v_r, 1, r), "v1" if r == NT - 1 else None)
        q(3.4, F(emit_qk_ct, 1, 1))
        q(3.4, F(emit_qk_ct, 1, 5), "qk11")
        emit_pair(1, 0)

        osb[1] = p_mid.tile([128, KD, N], F16, tag="mid", bufs=2, name="o1")

        force("v1")
        force("qk11")
        fillers.insert(0, (3.4, F(emit_burst, 1, 0), None))
        fillers.insert(1, (3.4, F(emit_burst, 1, 1), None))
        q(3.4, F(emit_qk_ct, 1, 2))
        q(3.4, F(emit_qk_ct, 1, 6), "qk12")
        q(1.9, F(emit_yproj_r, 0, 0))
        q(1.9, F(emit_yproj_r, 0, 1))
        emit_pair(1, 1)

        force("qk12")
        fillers.insert(0, (3.4, F(emit_burst, 1, 2), None))
        fillers.insert(1, (3.4, F(emit_burst, 1, 3), None))
        q(3.4, F(emit_qk_ct, 1, 3))
        q(3.4, F(emit_qk_ct, 1, 7), "qk13")
        for r in range(2, 5):
            q(1.9, F(emit_yproj_r, 0, r))
        emit_pair(1, 2)

        force("qk13")
        fillers.insert(0, (3.4, F(emit_burst, 1, 4), None))
        fillers.insert(1, (3.4, F(emit_burst, 1, 5), None))
        for r in range(5, 8):
            q(1.9, F(emit_yproj_r, 0, r))
        emit_pair(1, 3)
        drain_fillers()

        # Tail: last two heads with low-latency PE-broadcast normalize,
        # then the final output projection.
        emit_burst(1, 6, pe_norm=True)
        emit_burst(1, 7, pe_norm=True)
        for r in range(NT):
            emit_yproj_r(1, r)

    nc.compile()
    return nc


_NC = None


def _get_program():
    global _NC
    if _NC is None:
        _NC = build_program()
    return _NC


def make_in_maps(x, w_qkv, w_out, b_out, scale):
    x = np.ascontiguousarray(np.asarray(x, dtype=np.float32))
    w_qkv = np.ascontiguousarray(np.asarray(w_qkv, dtype=np.float32))
    w_out = np.ascontiguousarray(np.asarray(w_out, dtype=np.float32))
    b_out = np.ascontiguousarray(np.asarray(b_out, dtype=np.float32))
    scale = np.ascontiguousarray(np.asarray(scale, dtype=np.float32))
    return [
        {
            "x": x[c * BPC:(c + 1) * BPC],
            "w_qkv": w_qkv,
            "w_out": w_out,
            "b_out": b_out,
            "scale": scale,
        }
        for c in range(N_CORES)
    ]


def kernel(x, w_qkv, w_out, b_out, scale):
    nc = _get_program()
    in_maps = make_in_maps(x, w_qkv, w_out, b_out, scale)
    res = run_bass_kernel_spmd(nc, in_maps, core_ids=list(range(N_CORES)))
    return np.concatenate([res.results[c]["y"] for c in range(N_CORES)], axis=0)


if __name__ == "__main__":
    rng = np.random.default_rng(0)
    inputs = {
        "x": rng.standard_normal((B, N, D), dtype=np.float32),
        "w_qkv": rng.standard_normal((D, 3 * D), dtype=np.float32) * 0.03,
        "w_out": rng.standard_normal((D, D), dtype=np.float32) * 0.04,
        "b_out": np.zeros(D, dtype=np.float32),
        "scale": np.full(HEADS, DH ** -0.5, dtype=np.float32),
    }
    out = kernel(**inputs)
    print("kernel output", out.shape, out.dtype)
